# revision 1
# baseline (speedup 1.0000x reference)
"""Single-head attention kernel for Trainium2 (Bass/Tile), 8-core data-parallel.

Problem: h [8, 4096, 96] f32; Wq/Wk/Wv [96, 96]; bq/bk/bv [96].
  Q = h @ Wq.T + bq ; K = h @ Wk.T + bk ; V = h @ Wv.T + bv
  out = softmax(Q K^T / sqrt(96)) @ V

Sharding: batch dim across the 8 NeuronCores (1 batch element per core),
params replicated. Each core runs a flash-style attention over its
[4096, 96] slice; full output gathered on host.

Per-core layout (B=1, S=4096, D=96):
  - h staged in SBUF via 5 large DMAs (a small leading 4-tile slab, then
    [128, 8, 96] slabs rearranged from DRAM rows) instead of 32 small
    ones: DMA dispatch serialization and the 8-core HBM burst dominated
    the original 29us prologue.
  - h~^T [97, S] (row 96 = ones, written by a Pool-engine memset through a
    f32 bitcast view) so projection matmuls fold the bias add.
  - Q^T,K^T [96,S] f32r; V~ [128, nj, 97] with ones column (denominators).
  - h~^T / W~^T / the out^T epilogue run in bf16 (halved LDWEIGHTS and
    transpose stream time; the attention matmuls stay f32r). Adds ~0.5%
    relative error against a 2e-2 gate.
  - PSUM map: 3 rotating [128,1024] slots shared by scores and small
    transpose/projection tiles (banks 0-5; separate tiles keep deps
    fine-grained and give the PE ~3 tiles of runway ahead of ACT, which
    keeps the tensor engine p-state hot), banks 6-7 = accumulator.
  - Flat global loop over 128 (sweep, j) iterations: scores -> exp (ACT) ->
    PV (lagged), with accumulator copies and output epilogues deferred into
    the next sweep's iterations so neither PE nor ACT ever drains at sweep
    boundaries.  Output stores are batched into one DMA per sweep.
  - Epilogue: PE-transpose out^T [97,128] chunks (bf16), DVE reciprocal +
    scale, batched DMA (one per sweep).
  - Attention matmuls (scores QK^T, PV) in float32r (full PE rate at
    N=512 moving dim); h transposes, Q/K/V projections and the output
    epilogue in bf16 (halved LDWEIGHTS/stream cost off the critical path).
  - 4 exp tiles per sweep (sweeps 1+) run as a Schraudolph bitcast
    approximation on DVE with their PV deferred to the sweep end,
    unloading the Scalar engine which otherwise paces the loop; their
    j-positions avoid the early-sweep window where epilogue DVE work
    would delay the PSUM slot release. End-to-end rel err ~4e-3 against
    the 2e-2 gate. Measured ~173.5-175us on healthy silicon.

Identified next optimization (not implemented): fuse the V projection into
PV via out^T = W~v . (sum_j h~_j^T w_attn_j) -- accumulate M = sum_j
h~_j^T e_t_j (same PE cost as PV today, using raw h~ tiles as lhsT) and
apply W~v once per sweep (2 matmuls). Removes the 32 V-projection matmuls
and their DVE copies from the congested first sweep (~2.4us PE net, ~5us
DVE) at the cost of bf16 e_t and an epilogue reshuffle.
"""

import functools
import math

import numpy as np

import concourse.mybir as mybir
import concourse.tile as tile
from concourse import bacc
from concourse.bass import ts
from concourse.bass_utils import run_bass_kernel_spmd

S = 4096
D = 96
P = 128              # s-tile (partition) size
N_CORES = 8
F32 = mybir.dt.float32
F32R = mybir.dt.float32r
BF16 = mybir.dt.bfloat16
I32 = mybir.dt.int32
AF = mybir.ActivationFunctionType

# Schraudolph fast-exp: bitcast(int32(A*x + B)) ~= exp(x); C centers the
# mantissa-linear sawtooth (~1.8% rms on offloaded tiles). A few j-tiles
# per sweep run on DVE to unload the Scalar engine, which paces the steady
# state; their PV consumption is deferred to the sweep end so the slower
# DVE path never stalls the tensor engine.
SCH_A = float(2 ** 23 / math.log(2))
SCH_B = float(127 * 2 ** 23 - 486411)
# j tiles offloaded to DVE, placed outside the early-sweep window where
# the epilogue units' DVE ops would delay the Schraudolph read that frees
# the scores PSUM slot (late release there stalls the PE score pipeline).
OFF_JS = (11, 17, 23, 29)


def build_attention_kernel(tc, out_dram, h, Wq, bq, Wk, bk, Wv, bv, s=S):
    nc = tc.nc
    nj = s // P            # 32 j tiles (K/V position tiles)
    nsw = s // 1024        # 4 i-sweeps of 1024 columns
    G = nsw * nj           # 128 global iterations
    scale = 1.0 / math.sqrt(D)
    MMDT = F32R

    from contextlib import ExitStack
    with ExitStack() as ctx:
        singles = ctx.enter_context(tc.tile_pool(name="singles", bufs=1))
        tmp = ctx.enter_context(tc.tile_pool(name="tmp", bufs=8))
        expp = ctx.enter_context(tc.tile_pool(name="expp", bufs=9))
        schp = ctx.enter_context(tc.tile_pool(name="schp", bufs=2))
        epi = ctx.enter_context(tc.tile_pool(name="epi", bufs=2))
        outp = ctx.enter_context(tc.tile_pool(name="outp", bufs=2))
        osbp = ctx.enter_context(tc.tile_pool(name="osbp", bufs=2))
        # PSUM: 3 rotating slots (scores tiles and small transpose/
        # projection tiles share them -- separate tiles per slot keep the
        # dependency tracking fine-grained) + the accumulator = 8 banks.
        psp = ctx.enter_context(
            tc.tile_pool(name="psp", bufs=3, space="PSUM"))
        ps_accp = ctx.enter_context(
            tc.tile_pool(name="ps_acc", bufs=1, space="PSUM"))

        ident_dram = nc.inline_tensor(np.eye(P, dtype=np.float32),
                                      name="ident_const")

        # --- persistent SBUF tensors ---
        h_sb = singles.tile([P, nj, D], F32)      # staged h (row-major tiles)
        h_bf = singles.tile([P, nj, D], BF16)     # bf16 copy (cheap ldweights)
        hT = singles.tile([D + 1, s], BF16)       # h~^T (row 96 = ones)
        QT = singles.tile([D, s], MMDT)           # (Q + bq)^T / sqrt(D)
        KT = singles.tile([D, s], MMDT)
        Vt = singles.tile([P, nj, D + 1], MMDT)   # V~ tiles (col 96 = ones)
        ident = singles.tile([P, P], F32)

        # --- prologue DMAs ---
        # ident first (weight transposes need it immediately), then h in 4
        # big DMAs on the sync HWDGE queue. Each h DMA covers 8 j-tiles:
        # dst[p, t, e] = h[(8k+t)*128 + p, e].
        # First 4 h tiles as their own small DMA ahead of everything: all
        # 8 cores burst-load h simultaneously, so transfer time (HBM
        # contention), not dispatch, gates the first transposes.
        src0 = h[0:512, :].rearrange("(t p) e -> p t e", p=P)
        nc.sync.dma_start(out=h_sb[:, 0:4, :], in_=src0)
        nc.sync.dma_start(out=ident, in_=ident_dram.ap())
        src1 = h[512:1024, :].rearrange("(t p) e -> p t e", p=P)
        nc.sync.dma_start(out=h_sb[:, 4:8, :], in_=src1)
        for k in range(1, 4):
            src = h[k * 1024:(k + 1) * 1024, :].rearrange(
                "(t p) e -> p t e", p=P)
            nc.sync.dma_start(out=h_sb[:, 8 * k:8 * k + 8, :], in_=src)
        # DVE runs in emission order: cast only chunk 0 here (chunks 1-3
        # are cast inside the loop extras once their DMAs have landed), so
        # the first hT copies aren't queued behind casts stalled on DMAs.
        nc.vector.tensor_copy(h_bf[:, 0:4, :], h_sb[:, 0:4, :])
        nc.vector.tensor_copy(h_bf[:, 4:8, :], h_sb[:, 4:8, :])
        # weights on the scalar HWDGE queue (ACT idle during prologue).
        # K first: its ~650ns/dispatch serialization sits on the critical
        # chain to the first scores tile (kt0 needs the K transpose).
        w_sbs = []
        for W, b_ in ((Wk, bk), (Wq, bq), (Wv, bv)):
            w_sb = tmp.tile([D, D], F32, tag=f"w_sb{len(w_sbs)}")
            nc.scalar.dma_start(out=w_sb, in_=W)
            b_sb = tmp.tile([1, D], F32, tag=f"b_sb{len(w_sbs)}")
            nc.scalar.dma_start(out=b_sb, in_=b_.unsqueeze(0))
            w_sbs.append((w_sb, b_sb))
        # ones row / ones column via Pool engine (off the DVE critical path).
        # memset can't target f32r, so write through a f32 bitcast view.
        # Split the row so kt/qt chunk 0 aren't gated on the full 4096 cols.
        nc.gpsimd.memset(hT[D:D + 1, 0:1024], 1.0)
        nc.gpsimd.memset(hT[D:D + 1, 1024:s], 1.0)
        nc.gpsimd.memset(Vt[:, :, D].bitcast(F32), 1.0)

        # --- augmented transposed weights W~^T [97, 96] (row 96 = bias) ---
        def build_wt(i):
            w_sb, b_sb = w_sbs[i]
            ps_w = psp.tile([D, D], F32, tag="ps")
            nc.tensor.transpose(ps_w, w_sb, ident[0:D, 0:D])
            wt = singles.tile([D + 1, D], BF16, tag=f"wt{i}", name=f"wt{i}")
            nc.vector.tensor_copy(wt[0:D, :], ps_w)
            nc.vector.tensor_copy(wt[D:D + 1, :], b_sb)
            return wt

        wkt = build_wt(0)
        wqt = build_wt(1)
        wvt = None           # built after the first scores are in flight
        ident_bf = singles.tile([P, P], BF16)
        nc.vector.tensor_copy(ident_bf, ident)

        # --- emission helpers ---
        def emit_transpose(j):
            ps_t = psp.tile([D, P], BF16, tag="ps")
            nc.tensor.transpose(ps_t, h_bf[:, j, :], ident_bf)
            nc.vector.tensor_copy(hT[0:D, ts(j, P)], ps_t)

        def emit_qt_proj(n):
            ps_q = psp.tile([D, 512], F32, tag="ps")
            nc.tensor.matmul(ps_q, lhsT=wqt, rhs=hT[:, ts(n, 512)],
                             start=True, stop=True)
            nc.vector.tensor_scalar_mul(QT[:, ts(n, 512)], ps_q, scale)

        def emit_kt_proj(n):
            ps_k = psp.tile([D, 512], F32, tag="ps")
            nc.tensor.matmul(ps_k, lhsT=wkt, rhs=hT[:, ts(n, 512)],
                             start=True, stop=True)
            nc.vector.tensor_copy(KT[:, ts(n, 512)], ps_k)

        def emit_v_proj(j):
            ps_v = psp.tile([P, D], F32, tag="ps")
            nc.tensor.matmul(ps_v, lhsT=hT[:, ts(j, P)], rhs=wvt,
                             start=True, stop=True)
            nc.vector.tensor_copy(Vt[:, j, 0:D], ps_v)

        # --- prologue compute: minimum for g=0, rest interleaved ---
        # scores(g=0) needs hT tiles 0..7 (for QT cols 0..1023 and KT slab 0),
        # kt chunk 0, qt chunks 0-1.
        for j in range(4):
            emit_transpose(j)
        emit_kt_proj(0)
        emit_qt_proj(0)
        for j in range(4, 8):
            emit_transpose(j)
        emit_qt_proj(1)
        # V weight transpose deferred here: nothing needs it before the
        # first V projection, and ahead of the h transposes it would gate
        # the whole first-scores chain on the last weight DMA.
        wvt = build_wt(2)
        state = {"t": 8, "kt": 1, "qt": 2, "v": 0}

        def extras(g):
            # stage the remaining h bf16 casts now that their DMAs are in
            if g in (0, 2, 4):
                k = g // 2 + 1
                nc.vector.tensor_copy(h_bf[:, 8 * k:8 * k + 8, :],
                                      h_sb[:, 8 * k:8 * k + 8, :])
            # Interleaved prologue work, paced so dependencies stay ahead:
            # 2 transposes per iteration until done; kt chunk k once
            # transposes 4k+3 exist; v proj j at iteration j; later qt
            # chunks two per sweep.
            for _ in range(2):
                if state["t"] < nj:
                    emit_transpose(state["t"])
                    state["t"] += 1
            if state["kt"] < 8 and state["t"] >= 4 * state["kt"] + 4:
                emit_kt_proj(state["kt"])
                state["kt"] += 1
            if state["v"] < nj and state["v"] <= g:
                emit_v_proj(state["v"])
                state["v"] += 1
                # catch up V (it trails transposes early on)
                if state["v"] < min(g, nj) and state["t"] >= state["v"] + 2:
                    emit_v_proj(state["v"])
                    state["v"] += 1
            if state["qt"] < 2 * nsw and state["qt"] <= 2 * (g >> 5) + 3 \
                    and state["t"] >= nj:
                emit_qt_proj(state["qt"])
                state["qt"] += 1

        def scores_of(g):
            sw, j = g >> 5, g & 31
            i0 = sw * 1024
            ps_s = psp.tile([P, 1024], F32, tag="ps")
            for n in range(2):
                nc.tensor.matmul(
                    ps_s[:, ts(n, 512)],
                    lhsT=KT[:, ts(j, P)],
                    rhs=QT[:, i0 + 512 * n: i0 + 512 * (n + 1)],
                    start=True, stop=True)
            e_t = expp.tile([P, 1024], MMDT, tag="exp")
            if j in OFF_JS and g >= 32:
                sch = schp.tile([P, 1024], I32, tag="sch")
                nc.vector.tensor_scalar(
                    sch, ps_s, SCH_A, SCH_B,
                    mybir.AluOpType.mult, mybir.AluOpType.add)
                nc.vector.tensor_copy(e_t, sch.bitcast(F32))
            else:
                nc.scalar.activation(out=e_t, in_=ps_s, func=AF.Exp)
            return e_t

        def pv_of(g, e_t, acc, stop):
            sw, j = g >> 5, g & 31
            for n in range(2):
                nc.tensor.matmul(acc[:, ts(n, 512)], lhsT=Vt[:, j, :],
                                 rhs=e_t[:, ts(n, 512)],
                                 start=(j == 0), stop=stop)

        # --- epilogue machinery ---
        def emit_acc_copy(sw, oT, half, acc):
            # acc[:, half] -> oT[:, half]; two 256-col copies release the
            # accumulator banks earlier for the next sweep's first PV.
            nc.vector.tensor_copy(oT[:, half * 512:half * 512 + 256],
                                  acc[:, half * 512:half * 512 + 256])
            nc.vector.tensor_copy(oT[:, half * 512 + 256:half * 512 + 512],
                                  acc[:, half * 512 + 256:half * 512 + 512])

        def emit_epilogue_unit(sw, oT, c, o_sb):
            ps_tr = psp.tile([P, D + 1], BF16, tag="ps")
            nc.tensor.transpose(ps_tr, oT[:, ts(c, P)],
                                ident_bf[0:D + 1, 0:D + 1])
            rec = outp.tile([P, 1], F32, tag="rec")
            nc.vector.reciprocal(rec, ps_tr[:, D:D + 1])
            nc.vector.tensor_scalar_mul(o_sb[:, c, :], ps_tr[:, 0:D], rec)

        def emit_out_dma(sw, o_sb, half):
            r0 = sw * 1024 + half * 512
            dst = out_dram[r0:r0 + 512, :].rearrange("(u p) e -> p u e", p=P)
            nc.sync.dma_start(out=dst, in_=o_sb[:, 4 * half:4 * half + 4, :])

        # --- flat main loop ---
        LAG = 3
        exp_tiles = [None] * G
        acc_of = {}      # sweep -> accumulator tile
        deferred = {}    # sweep -> offloaded-PV iterations
        oT_of = {}
        osb_of = {}
        pending = []     # deferred epilogue closures

        def get_acc(sw):
            if sw not in acc_of:
                acc_of[sw] = ps_accp.tile([D + 1, 1024], F32, tag="acc", name="acc")
            return acc_of[sw]

        def do_pv(gp):
            # offloaded tiles (DVE exp) are consumed at the sweep end so
            # their extra latency never bubbles the tensor engine; the last
            # deferred PV carries the accumulation-group stop flag.
            swp, jp = gp >> 5, gp & 31
            offloaded = jp in OFF_JS and gp >= 32
            if offloaded:
                deferred.setdefault(swp, []).append(gp)
                return
            stop = (jp == nj - 1) and gp < 32
            pv_of(gp, exp_tiles[gp], get_acc(swp), stop)
            exp_tiles[gp] = None

        def flush_deferred(swp):
            lst = deferred.pop(swp, [])
            for i, gd in enumerate(lst):
                pv_of(gd, exp_tiles[gd], get_acc(swp), i == len(lst) - 1)
                exp_tiles[gd] = None

        for g in range(G):
            exp_tiles[g] = scores_of(g)
            extras(g)
            if pending and (g & 31) >= 2:
                pending.pop(0)()
                if pending and (g & 31) >= 6:
                    pending.pop(0)()
            if g >= LAG:
                gp = g - LAG
                do_pv(gp)
                if (gp & 31) == 31:
                    flush_deferred(gp >> 5)
                    # sweep gp>>5 fully accumulated: queue its epilogue
                    swd = gp >> 5
                    oT = epi.tile([D + 1, 1024], BF16, tag="oT")
                    o_sb = osbp.tile([P, 8, D], F32, tag="o_sb")
                    oT_of[swd], osb_of[swd] = oT, o_sb
                    acc = acc_of.pop(swd)
                    # copies are DVE-only: emit immediately so the
                    # accumulator banks free before the next sweep's first
                    # PV, without perturbing the PE instruction stream.
                    emit_acc_copy(swd, oT, 0, acc)
                    emit_acc_copy(swd, oT, 1, acc)
                    for c in range(8):
                        pending.append(
                            lambda sw=swd, t=oT, c=c, o=osb_of[swd]:
                            emit_epilogue_unit(sw, t, c, o))
                        if c == 3:
                            pending.append(
                                lambda sw=swd, o=o_sb: emit_out_dma(sw, o, 0))
                    pending.append(
                        lambda sw=swd, o=o_sb: emit_out_dma(sw, o, 1))

        # drain PV tail and remaining epilogues
        for g in range(G - LAG, G):
            do_pv(g)
            if (g & 31) == 31:
                flush_deferred(g >> 5)
                swd = g >> 5
                oT = epi.tile([D + 1, 1024], BF16, tag="oT")
                o_sb = osbp.tile([P, 8, D], F32, tag="o_sb")
                acc = acc_of.pop(swd)
                emit_acc_copy(swd, oT, 0, acc)
                emit_acc_copy(swd, oT, 1, acc)
                for c in range(8):
                    emit_epilogue_unit(swd, oT, c, o_sb)
                    if c == 3:
                        emit_out_dma(swd, o_sb, 0)
                emit_out_dma(swd, o_sb, 1)
        while pending:
            pending.pop(0)()


@functools.lru_cache(maxsize=None)
def _build_module(s=S):
    nc = bacc.Bacc("TRN2", target_bir_lowering=False, debug=False,
                   num_devices=N_CORES)
    h = nc.dram_tensor("h", [s, D], F32, kind="ExternalInput").ap()
    Wq = nc.dram_tensor("Wq", [D, D], F32, kind="ExternalInput").ap()
    bq = nc.dram_tensor("bq", [D], F32, kind="ExternalInput").ap()
    Wk = nc.dram_tensor("Wk", [D, D], F32, kind="ExternalInput").ap()
    bk = nc.dram_tensor("bk", [D], F32, kind="ExternalInput").ap()
    Wv = nc.dram_tensor("Wv", [D, D], F32, kind="ExternalInput").ap()
    bv = nc.dram_tensor("bv", [D], F32, kind="ExternalInput").ap()
    out = nc.dram_tensor("out", [s, D], F32, kind="ExternalOutput").ap()
    with tile.TileContext(nc) as tc:
        build_attention_kernel(tc, out, h, Wq, bq, Wk, bk, Wv, bv, s=s)
    nc.compile()
    return nc


def _run(inputs, trace=False):
    nc = _build_module(S)
    arrs = {k: np.ascontiguousarray(np.asarray(v), dtype=np.float32)
            for k, v in inputs.items()}
    in_maps = []
    for b_ in range(N_CORES):
        in_maps.append({
            "h": arrs["h"][b_],
            "Wq": arrs["Wq"], "bq": arrs["bq"],
            "Wk": arrs["Wk"], "bk": arrs["bk"],
            "Wv": arrs["Wv"], "bv": arrs["bv"],
        })
    res = run_bass_kernel_spmd(nc, in_maps, core_ids=list(range(N_CORES)),
                               trace=trace)
    out = np.stack([res.results[b_]["out"] for b_ in range(N_CORES)], axis=0)
    return out, res


def kernel(**inputs):
    out, _ = _run(inputs, trace=False)
    return out


def kernel_profiled(trace=True, **inputs):
    out, res = _run(inputs, trace=trace)
    return out, res

